# revision 1
# baseline (speedup 1.0000x reference)
"""Trainium2 Bass kernel for nn_Attention_84585085927925 — Gram variant.

Reference (per batch element b, all fp32):
    qkv = x @ w_qkv.T ; q,k,v heads of 64 ; attn = sqrt(64) * q @ k.T (NO
    softmax) ; out = attn @ v ; out = out @ w_fc.T + b_fc

With no softmax the attention is associative, and k/v can be folded into
the weights via the Gram matrix of x:
    out_h = (s*q_h) @ (k_h.T @ v_h) = (s*q_h) @ (wk_h @ (x.T x) @ wv_h.T)
Per-core pipeline (one batch element per NeuronCore, 8 cores, no
collectives; e = output-feature axis, d = input-feature axis):
    qT   = (s*w_q).T-stationary @ xT       -> [768,1024]
    C    = x.T x  (x-stationary)           -> [768,768] (symmetric)
    T1   = C-stationary @ wv.T             -> [768,768]
    G    = wk.T-stationary @ T1 per pair   -> block-diag [128,128] per pair
    aoT  = G2-stationary @ qT per pair     -> [768,1024]
    outT = w_fc.T-stationary @ aoT + b_fc  -> [768,1024]
Host transposes x and outT. Matmuls run in float32r (~4x faster than
fp32, ~3e-4 relative error).
"""

import numpy as np

import concourse.bass as bass  # noqa: F401  (registers engine namespaces)
import concourse.mybir as mybir
import concourse.tile as tile
from concourse import bacc, bass_utils

F32 = mybir.dt.float32
F32R = mybir.dt.float32r

B, N, D, H = 8, 1024, 768, 12
HD = D // H            # 64
SCALE = float(np.sqrt(HD))
DT = D // 128          # 6  d-tiles
ET = D // 128          # 6  e-tiles
NT = N // 128          # 8  n(token)-tiles
NC2 = N // 512         # 2  512-wide token chunks
ECH = 384              # e-chunk that fits one PSUM bank with headroom
NPAIR = H // 2         # 6 head pairs


def _build_program():
    nc = bacc.Bacc(
        trn_type="TRN2", target_bir_lowering=False, debug=False, num_devices=B
    )
    xT_d = nc.dram_tensor("xT", [D, N], F32, kind="ExternalInput").ap()
    xN_d = nc.dram_tensor("xN", [N, D], F32, kind="ExternalInput").ap()
    wqT_d = nc.dram_tensor("wqT", [D, D], F32, kind="ExternalInput").ap()
    wkT_d = nc.dram_tensor("wkT", [D, D], F32, kind="ExternalInput").ap()
    wvT_d = nc.dram_tensor("wvT", [D, D], F32, kind="ExternalInput").ap()
    wfcT_d = nc.dram_tensor("wfcT", [D, D], F32, kind="ExternalInput").ap()
    bfc_d = nc.dram_tensor("bfc", [D], F32, kind="ExternalInput").ap()
    outT_d = nc.dram_tensor("outT", [D, N], F32, kind="ExternalOutput").ap()

    with tile.TileContext(nc) as tc:
        with tc.tile_pool(name="big", bufs=1) as big, \
             tc.tile_pool(name="wsp", bufs=3) as wsp, \
             tc.tile_pool(name="outsp", bufs=6) as outsp, \
             tc.tile_pool(name="ps", bufs=6, space="PSUM") as ps, \
             tc.tile_pool(name="psg", bufs=2, space="PSUM") as psg:

            xT_sb = big.tile([128, DT, N], F32R, name="xT_sb")
            xN_sb = big.tile([128, NT, D], F32R, name="xN_sb")
            qT_sb = big.tile([128, ET, N], F32R, name="qT_sb")
            c_sb = big.tile([128, DT, D], F32R, name="c_sb")
            t1_sb = big.tile([128, DT, D], F32R, name="t1_sb")
            ao_sb = big.tile([128, DT, N], F32R, name="ao_sb")
            g2_sb = big.tile([128, NPAIR, 128], F32R, name="g2_sb")
            bias_sb = big.tile([128, ET], F32, name="bias_sb")

            wq_r = wqT_d.rearrange("(o p) e -> p o e", p=128).bitcast(F32R)
            xT_r = xT_d.rearrange("(o p) n -> p o n", p=128).bitcast(F32R)
            xN_r = xN_d.rearrange("(o p) e -> p o e", p=128).bitcast(F32R)

            wq_tiles = []
            for et in range(ET):
                wq_t = wsp.tile([128, DT, 128], F32R, tag="w128", bufs=7,
                                name=f"wq_t{et}", uniquify=False)
                wq_tiles.append(wq_t)
            # first-needed data first: wq0 halves, xT n-half 0, then the rest
            for dh in range(2):
                dsl = slice(dh * 3, dh * 3 + 3)
                nc.sync.dma_start(wq_tiles[0][:, dsl, :], wq_r[:, dsl, 0:128])
            for dt in range(DT):
                nc.sync.dma_start(xT_sb[:, dt, 0:512], xT_r[:, dt, 0:512])
            for et in range(1, ET):
                for dh in range(2):
                    dsl = slice(dh * 3, dh * 3 + 3)
                    nc.sync.dma_start(wq_tiles[et][:, dsl, :],
                                      wq_r[:, dsl, et * 128:(et + 1) * 128])
            for dt in range(DT):
                nc.sync.dma_start(xT_sb[:, dt, 512:1024], xT_r[:, dt, 512:1024])
            for nt in range(NT):
                nc.sync.dma_start(xN_sb[:, nt, :], xN_r[:, nt, :])
            nc.sync.dma_start(bias_sb[:],
                              bfc_d.rearrange("(o p) -> p o", p=128))

            # ---- q.T projection: lhsT = wqT tile [d,e], rhs = xT chunk ----
            qt_chunks = [(0, 0, 256), (0, 256, 256),
                         (1, 0, 512), (2, 0, 512), (3, 0, 512),
                         (4, 0, 512), (5, 0, 512),
                         (0, 512, 512), (1, 512, 512), (2, 512, 512),
                         (3, 512, 512), (4, 512, 512), (5, 512, 512)]
            for et, off, width in qt_chunks:
                wq_t = wq_tiles[et]
                pt = ps.tile([128, 512], F32, tag="ps", name="pt_q")
                for dt in range(DT):
                    nc.tensor.matmul(
                        pt[:, :width],
                        wq_t[:, dt, :],
                        xT_sb[:, dt, off:off + width],
                        start=(dt == 0), stop=(dt == DT - 1),
                    )
                nc.vector.tensor_copy(
                    qT_sb[:, et, off:off + width], pt[:, :width]
                )

            # ---- C = x.T x : lhsT = x tile [n, d1], rhs = x [n, d2-chunk] --
            for ec in range(D // ECH):
                for d1t in range(DT):
                    pt = ps.tile([128, ECH], F32, tag="ps", name="pt_c")
                    for nt in range(NT):
                        nc.tensor.matmul(
                            pt[:],
                            xN_sb[:, nt, d1t * 128:(d1t + 1) * 128],
                            xN_sb[:, nt, ec * ECH:(ec + 1) * ECH],
                            start=(nt == 0), stop=(nt == NT - 1),
                        )
                    nc.vector.tensor_copy(
                        c_sb[:, d1t, ec * ECH:(ec + 1) * ECH], pt[:]
                    )

            # ---- T1 = C @ wv.T : lhsT = C tile (symmetric), rhs = wvT ----
            wv_r = wvT_d.rearrange("(o p) e -> p o e", p=128).bitcast(F32R)
            for ec in range(D // ECH):
                wv_t = wsp.tile([128, DT, ECH], F32R, tag="w384",
                                name=f"wv_t{ec}", uniquify=False)
                for dh in range(3):
                    dsl = slice(dh * 2, dh * 2 + 2)
                    nc.sync.dma_start(
                        wv_t[:, dsl, :],
                        wv_r[:, dsl, ec * ECH:(ec + 1) * ECH],
                    )
                for d1t in range(DT):
                    pt = ps.tile([128, ECH], F32, tag="ps", name="pt_t1")
                    for d2t in range(DT):
                        nc.tensor.matmul(
                            pt[:],
                            c_sb[:, d2t, d1t * 128:(d1t + 1) * 128],
                            wv_t[:, d2t, :],
                            start=(d2t == 0), stop=(d2t == DT - 1),
                        )
                    nc.vector.tensor_copy(
                        t1_sb[:, d1t, ec * ECH:(ec + 1) * ECH], pt[:]
                    )

            # ---- G = wk @ T1 per head pair, stored block-diagonal ----
            wk_r = wkT_d.rearrange("(o p) e -> p o e", p=128).bitcast(F32R)
            for t in range(NPAIR):
                wk_t = wsp.tile([128, DT, 128], F32R, tag="w128", bufs=7,
                                name=f"wk_t{t}", uniquify=False)
                nc.sync.dma_start(wk_t[:], wk_r[:, :, t * 128:(t + 1) * 128])
                gp = psg.tile([128, 128], F32, tag="psg", name="gp")
                for dt in range(DT):
                    nc.tensor.matmul(
                        gp[:],
                        wk_t[:, dt, :],
                        t1_sb[:, dt, t * 128:(t + 1) * 128],
                        start=(dt == 0), stop=(dt == DT - 1),
                    )
                nc.vector.tensor_scalar_mul(g2_sb[:, t, :], gp[:], 0.0)
                nc.vector.tensor_copy(g2_sb[0:64, t, 0:64], gp[0:64, 0:64])
                nc.vector.tensor_copy(g2_sb[64:128, t, 64:128],
                                      gp[64:128, 64:128])

            # ---- attn-out.T then fc, interleaved per 512-chunk ----
            wfc_r = wfcT_d.rearrange("(o p) e -> p o e", p=128).bitcast(F32R)
            wfc_tiles = []
            for et in range(ET):
                wfc_t = wsp.tile([128, DT, 128], F32R, tag="w128", bufs=7,
                                 name=f"wfc_t{et}", uniquify=False)
                wfc_tiles.append(wfc_t)
                nc.sync.dma_start(wfc_t[:], wfc_r[:, :, et * 128:(et + 1) * 128])
            for ic in range(NC2):
                for t in range(NPAIR):
                    pt = ps.tile([128, 512], F32, tag="ps", name="pt_ao")
                    nc.tensor.matmul(
                        pt[:],
                        g2_sb[:, t, :],
                        qT_sb[:, t, ic * 512:(ic + 1) * 512],
                        start=True, stop=True,
                    )
                    dst_ap = ao_sb[:, t, ic * 512:(ic + 1) * 512]
                    if t % 2 == 0:
                        nc.vector.tensor_copy(dst_ap, pt[:])
                    else:
                        nc.scalar.copy(dst_ap, pt[:])
                for et in range(ET):
                    wfc_t = wfc_tiles[et]
                    pt = ps.tile([128, 512], F32, tag="ps", name="pt_fc")
                    for dt in range(DT):
                        nc.tensor.matmul(
                            pt[:],
                            wfc_t[:, dt, :],
                            ao_sb[:, dt, ic * 512:(ic + 1) * 512],
                            start=(dt == 0), stop=(dt == DT - 1),
                        )
                    ot = outsp.tile([128, 512], F32, tag="ot", name="ot")
                    nc.scalar.add(ot[:], pt[:], bias_sb[:, et:et + 1])
                    last = (ic == NC2 - 1 and et == ET - 1)
                    nsplit = 4 if last else 1
                    w = 128 // nsplit
                    for ph in range(nsplit):
                        nc.sync.dma_start(
                            outT_d[et * 128 + ph * w:et * 128 + (ph + 1) * w,
                                   ic * 512:(ic + 1) * 512],
                            ot[ph * w:(ph + 1) * w, :],
                        )

    nc.compile()
    return nc


_NC_CACHE = None
LAST_EXEC_NS = None
LAST_RES = None


def kernel(x, w_qkv, w_fc, b_fc, _trace=False):
    global _NC_CACHE, LAST_EXEC_NS, LAST_RES
    x = np.asarray(x, dtype=np.float32)
    w_qkv = np.asarray(w_qkv, dtype=np.float32)
    w_fc = np.asarray(w_fc, dtype=np.float32)
    b_fc = np.asarray(b_fc, dtype=np.float32)

    if _NC_CACHE is None:
        _NC_CACHE = _build_program()
    nc = _NC_CACHE

    wqT = np.ascontiguousarray((SCALE * w_qkv[:D]).T)
    wkT = np.ascontiguousarray(w_qkv[D:2 * D].T)
    wvT = np.ascontiguousarray(w_qkv[2 * D:].T)
    wfcT = np.ascontiguousarray(w_fc.T)

    in_maps = []
    for b in range(B):
        in_maps.append({
            "xT": np.ascontiguousarray(x[b].T),
            "xN": np.ascontiguousarray(x[b]),
            "wqT": wqT, "wkT": wkT, "wvT": wvT, "wfcT": wfcT,
            "bfc": b_fc,
        })

    res = bass_utils.run_bass_kernel_spmd(
        nc, in_maps, core_ids=list(range(B)), trace=_trace
    )
    LAST_EXEC_NS = res.exec_time_ns
    LAST_RES = res
    out = np.stack([res.results[b]["outT"].T for b in range(B)])
    return np.ascontiguousarray(out.astype(np.float32))



# revision 6
# speedup vs baseline: 1.3680x; 1.3680x over previous
"""Trainium2 Bass kernel for nn_Attention_84585085927925 — bf16 M-folded Gram.

Reference (per batch element b, all fp32):
    qkv = x @ w_qkv.T ; q,k,v heads of 64 ; attn = sqrt(64) * q @ k.T (NO
    softmax) ; out = attn @ v ; out = out @ w_fc.T + b_fc

No softmax => attention is linear; fold k/v AND the q/fc projections into a
single per-batch effective matrix M:
    out = x @ M + b_fc,   M = s * sum_h wq_h.T G_h wfc[:,h].T,
    G_h = wk_h C wv_h.T,  C = x.T x  (symmetric)
Per-core pipeline (one batch element per NeuronCore, 8 cores):
    C    upper-triangle blocks via PSUM, mirrored by PE transpose
    T1   = C @ wv.T                       [768,768]
    G    = wk_pair @ T1[:,pair]  (block-diag per head pair)
    M1T  = G_blkdiag.T @ (s*wq)_pair      [128,768] per pair
    M    = M1T.T @ wfc.T                  [768,768]
    outT = M.T @ xT + b_fc                [768,1024]
All matmuls bf16 (1 cyc/row, FWL weight loads); fp32 PSUM accumulate.
"""

import numpy as np
import ml_dtypes

import concourse.bass as bass  # noqa: F401  (registers engine namespaces)
import concourse.mybir as mybir
import concourse.tile as tile
from concourse import bacc, bass_utils

F32 = mybir.dt.float32
BF16 = mybir.dt.bfloat16
U32 = mybir.dt.uint32

B, N, D, H = 8, 1024, 768, 12
HD = D // H            # 64
SCALE = float(np.sqrt(HD))
DT = D // 128           # 6 blocks of 128 along feature dims
NT = N // 128           # 8 token tiles


def _build_program(debug_dumps=False, num_devices=B):
    nc = bacc.Bacc(
        trn_type="TRN2", target_bir_lowering=False, debug=False,
        num_devices=num_devices
    )
    xN_d = nc.dram_tensor("xN", [N, D], BF16, kind="ExternalInput").ap()
    xT_d = nc.dram_tensor("xT", [D, N], BF16, kind="ExternalInput").ap()
    wvT_d = nc.dram_tensor("wvT", [D, D], BF16, kind="ExternalInput").ap()
    wkT_d = nc.dram_tensor("wkT", [D, D], BF16, kind="ExternalInput").ap()
    wq_d = nc.dram_tensor("wq", [D, D], BF16, kind="ExternalInput").ap()
    wfcT_d = nc.dram_tensor("wfcT", [D, D], BF16, kind="ExternalInput").ap()
    bfc_d = nc.dram_tensor("bfc", [D], F32, kind="ExternalInput").ap()
    id_d = nc.dram_tensor("ident", [128, 128], BF16, kind="ExternalInput").ap()
    outT_d = nc.dram_tensor("outT", [D, N], BF16, kind="ExternalOutput").ap()
    dbg = {}
    if debug_dumps:
        for nm, shape in [("d_c", [128, DT, D]), ("d_t1", [128, DT, D]),
                          ("d_g2", [128, DT, 128]), ("d_m1t", [128, DT, D]),
                          ("d_m", [128, DT, D])]:
            dbg[nm] = nc.dram_tensor(nm, shape, BF16,
                                     kind="ExternalOutput").ap()

    with tile.TileContext(nc) as tc:
        with tc.tile_pool(name="big", bufs=1) as big, \
             tc.tile_pool(name="outsp", bufs=3) as outsp, \
             tc.tile_pool(name="psa", bufs=4, space="PSUM") as psa, \
             tc.tile_pool(name="psw", bufs=2, space="PSUM") as psw, \
             tc.tile_pool(name="psg", bufs=2, space="PSUM") as psg:

        # PSUM static budget: psa 4 banks + psw 2 + psg 2 = 8.

            xn_t = [big.tile([128, D], BF16, name=f"xn{o}") for o in range(NT)]
            xT_sb = big.tile([128, DT, N], BF16, name="xT_sb")
            wvT_sb = big.tile([128, DT, D], BF16, name="wvT_sb")
            wkT_sb = big.tile([128, DT, D], BF16, name="wkT_sb")
            wq_sb = big.tile([128, DT, D], BF16, name="wq_sb")
            wfcT_sb = big.tile([128, DT, D], BF16, name="wfcT_sb")
            c_sb = big.tile([128, DT, D], BF16, name="c_sb")
            t1_sb = big.tile([128, DT, D], BF16, name="t1_sb")
            g2_sb = big.tile([128, DT, 128], BF16, name="g2_sb")
            m1t_sb = big.tile([128, DT, D], BF16, name="m1t_sb")
            m_sb = big.tile([128, DT, D], BF16, name="m_sb")
            bias_sb = big.tile([128, DT], F32, name="bias_sb")
            id_sb = big.tile([128, 128], BF16, name="id_sb")

            xN_r = xN_d.rearrange("(o p) e -> p o e", p=128)
            xT_r = xT_d.rearrange("(o p) n -> p o n", p=128)
            wvT_r = wvT_d.rearrange("(o p) e -> p o e", p=128)
            wkT_r = wkT_d.rearrange("(o p) e -> p o e", p=128)
            wq_r = wq_d.rearrange("(o p) e -> p o e", p=128)
            wfcT_r = wfcT_d.rearrange("(o p) e -> p o e", p=128)
            outT_r = outT_d.rearrange("(o p) n -> p o n", p=128)

            # ---- DMA in on two hwdge queues (sync + scalar) ----
            # sync queue: xN even tiles first (C starts immediately)
            # scalar queue: xN odd tiles, then remaining weights
            nc.sync.dma_start(xn_t[0][:], xN_r[:, 0, :])
            nc.scalar.dma_start(xn_t[1][:], xN_r[:, 1, :])
            nc.sync.dma_start(xn_t[2][:], xN_r[:, 2, :])
            nc.scalar.dma_start(xn_t[3][:], xN_r[:, 3, :])
            nc.sync.dma_start(xn_t[4][:], xN_r[:, 4, :])
            nc.scalar.dma_start(xn_t[5][:], xN_r[:, 5, :])
            nc.sync.dma_start(xn_t[6][:], xN_r[:, 6, :])
            nc.scalar.dma_start(xn_t[7][:], xN_r[:, 7, :])
            nc.sync.dma_start(id_sb[:], id_d)
            nc.sync.dma_start(bias_sb[:], bfc_d.rearrange("(o p) -> p o", p=128))
            for h in range(2):
                sl = slice(h * 3, h * 3 + 3)
                nc.sync.dma_start(wvT_sb[:, sl, :], wvT_r[:, sl, :])
                nc.scalar.dma_start(wkT_sb[:, sl, :], wkT_r[:, sl, :])
                nc.sync.dma_start(wq_sb[:, sl, :], wq_r[:, sl, :])
                nc.scalar.dma_start(wfcT_sb[:, sl, :], wfcT_r[:, sl, :])
                nc.sync.dma_start(xT_sb[:, sl, :], xT_r[:, sl, :])

            # zero g2 once (gpsimd — off critical path)
            nc.gpsimd.memset(g2_sb[:], 0.0)

            copy_engines = [nc.vector.tensor_copy, nc.scalar.copy]
            ce_idx = [0]

            def copy(dst, src):
                copy_engines[ce_idx[0] % 2](dst, src)
                ce_idx[0] += 1

            # ---- C = x.T x, upper triangle, 3 phases of 2 rows ----
            # row r covers cols r*128:768 (split into <=512 chunks)
            row_chunks = {0: [(0, 512), (512, 256)], 1: [(128, 512), (640, 128)],
                          2: [(256, 512)], 3: [(384, 384)],
                          4: [(512, 256)], 5: [(640, 128)]}

            def c_phase(rows):
                tiles = []
                for r in rows:
                    for c0, w in row_chunks[r]:
                        pt = psa.tile([128, 512], F32, tag="a", name=f"c{r}_{c0}")
                        tiles.append((r, c0, w, pt))
                for nt in range(NT):
                    for r, c0, w, pt in tiles:
                        nc.tensor.matmul(
                            pt[:, :w],
                            xn_t[nt][:, r * 128:(r + 1) * 128],
                            xn_t[nt][:, c0:c0 + w],
                            start=(nt == 0), stop=(nt == NT - 1),
                        )
                for r, c0, w, pt in tiles:
                    copy(c_sb[:, r, c0:c0 + w], pt[:, :w])

            def emit_mirror(i, j):
                # slot (j, i) := transpose of stored upper block (i, j)
                tp = psg.tile([128, 128], BF16, tag="g", name=f"tr{i}{j}")
                nc.tensor.transpose(
                    tp[:], c_sb[:, i, j * 128:(j + 1) * 128], id_sb[:]
                )
                nc.vector.tensor_copy(
                    c_sb[:, j, i * 128:(i + 1) * 128].bitcast(U32),
                    tp[:].bitcast(U32))

            c_phase([0, 1])
            c_phase([2, 3])
            for i, j in [(0, 1), (0, 2), (0, 3), (0, 4), (0, 5),
                         (1, 2), (1, 3), (1, 4), (1, 5)]:
                emit_mirror(i, j)
            c_phase([4, 5])
            for i, j in [(2, 3), (2, 4), (2, 5), (3, 4), (3, 5), (4, 5)]:
                emit_mirror(i, j)

            # ---- T1 = C @ wv.T ----
            for i in range(DT):
                ua = psa.tile([128, 512], F32, tag="a", name=f"t1a{i}")
                ub = psa.tile([128, 512], F32, tag="a", name=f"t1b{i}")
                for j in range(DT):
                    lhsT = c_sb[:, j, i * 128:(i + 1) * 128]
                    nc.tensor.matmul(ua[:, :512], lhsT, wvT_sb[:, j, 0:512],
                                     start=(j == 0), stop=(j == DT - 1))
                    nc.tensor.matmul(ub[:, :256], lhsT, wvT_sb[:, j, 512:768],
                                     start=(j == 0), stop=(j == DT - 1))
                copy(t1_sb[:, i, 0:512], ua[:, :512])
                copy(t1_sb[:, i, 512:768], ub[:, :256])

            # ---- G (block-diag per pair) + M1T + M, software-pipelined ----
            def g_stage(t):
                gp = psg.tile([128, 128], F32, tag="g", name=f"g{t}")
                for j in range(DT):
                    nc.tensor.matmul(
                        gp[:],
                        wkT_sb[:, j, t * 128:(t + 1) * 128],
                        t1_sb[:, j, t * 128:(t + 1) * 128],
                        start=(j == 0), stop=(j == DT - 1),
                    )
                copy(g2_sb[0:64, t, 0:64], gp[0:64, 0:64])
                copy(g2_sb[64:128, t, 64:128], gp[64:128, 64:128])

            def m1t_stage(t):
                pa = psw.tile([128, 512], F32, tag="w", name=f"m1a{t}")
                pb = psw.tile([128, 512], F32, tag="w", name=f"m1b{t}")
                nc.tensor.matmul(pa[:, :512], g2_sb[:, t, :],
                                 wq_sb[:, t, 0:512], start=True, stop=True)
                nc.tensor.matmul(pb[:, :256], g2_sb[:, t, :],
                                 wq_sb[:, t, 512:768], start=True, stop=True)
                copy(m1t_sb[:, t, 0:512], pa[:, :512])
                copy(m1t_sb[:, t, 512:768], pb[:, :256])

            # M accumulators for dblocks 0,1 live across the G/M1T pipeline
            mtiles01 = []
            for db in (0, 1):
                ma = psa.tile([128, 512], F32, tag="a", name=f"ma{db}")
                mb = psa.tile([128, 512], F32, tag="a", name=f"mb{db}")
                mtiles01.append((db, ma, mb))

            def m_step(dbtiles, t):
                for db, ma, mb in dbtiles:
                    lhsT = m1t_sb[:, t, db * 128:(db + 1) * 128]
                    nc.tensor.matmul(ma[:, :512], lhsT, wfcT_sb[:, t, 0:512],
                                     start=(t == 0), stop=(t == DT - 1))
                    nc.tensor.matmul(mb[:, :256], lhsT, wfcT_sb[:, t, 512:768],
                                     start=(t == 0), stop=(t == DT - 1))

            g_stage(0)
            g_stage(1)
            m1t_stage(0)
            g_stage(2)
            m1t_stage(1)
            m_step(mtiles01, 0)
            g_stage(3)
            m1t_stage(2)
            m_step(mtiles01, 1)
            g_stage(4)
            m1t_stage(3)
            m_step(mtiles01, 2)
            g_stage(5)
            m1t_stage(4)
            m_step(mtiles01, 3)
            m1t_stage(5)
            m_step(mtiles01, 4)
            m_step(mtiles01, 5)
            for db, ma, mb in mtiles01:
                copy(m_sb[:, db, 0:512], ma[:, :512])
                copy(m_sb[:, db, 512:768], mb[:, :256])
            for phase in (1, 2):
                mts = []
                for db in (phase * 2, phase * 2 + 1):
                    ma = psa.tile([128, 512], F32, tag="a", name=f"ma{db}")
                    mb = psa.tile([128, 512], F32, tag="a", name=f"mb{db}")
                    mts.append((db, ma, mb))
                for t in range(DT):
                    m_step(mts, t)
                for db, ma, mb in mts:
                    copy(m_sb[:, db, 0:512], ma[:, :512])
                    copy(m_sb[:, db, 512:768], mb[:, :256])

            # ---- outT = M.T @ xT + bias ----
            for et in range(DT):
                oa = psa.tile([128, 512], F32, tag="a", name=f"oa{et}")
                ob = psa.tile([128, 512], F32, tag="a", name=f"ob{et}")
                for dt in range(DT):
                    lhsT = m_sb[:, dt, et * 128:(et + 1) * 128]
                    nc.tensor.matmul(oa[:, :512], lhsT, xT_sb[:, dt, 0:512],
                                     start=(dt == 0), stop=(dt == DT - 1))
                    nc.tensor.matmul(ob[:, :512], lhsT, xT_sb[:, dt, 512:1024],
                                     start=(dt == 0), stop=(dt == DT - 1))
                ot = outsp.tile([128, N], BF16, tag="ot", name=f"ot{et}")
                if et < DT - 1:
                    nc.scalar.add(ot[:, 0:512], oa[:, :512],
                                  bias_sb[:, et:et + 1])
                    nc.vector.tensor_scalar_add(ot[:, 512:1024], ob[:, :512],
                                                bias_sb[:, et:et + 1])
                    nc.sync.dma_start(outT_r[:, et, :], ot[:])
                else:
                    # last tile: stream halves to shorten the tail
                    nc.scalar.add(ot[:, 0:512], oa[:, :512],
                                  bias_sb[:, et:et + 1])
                    nc.sync.dma_start(outT_r[:, et, 0:512], ot[:, 0:512])
                    nc.vector.tensor_scalar_add(ot[:, 512:1024], ob[:, :512],
                                                bias_sb[:, et:et + 1])
                    nc.scalar.dma_start(outT_r[:, et, 512:1024],
                                        ot[:, 512:1024])

            if debug_dumps:
                for nm, sb in [("d_c", c_sb), ("d_t1", t1_sb),
                               ("d_g2", g2_sb), ("d_m1t", m1t_sb),
                               ("d_m", m_sb)]:
                    nc.sync.dma_start(dbg[nm], sb)

    nc.compile()
    return nc


_NC_CACHE = None
LAST_EXEC_NS = None
LAST_RES = None


def kernel(x, w_qkv, w_fc, b_fc, _trace=False):
    global _NC_CACHE, LAST_EXEC_NS, LAST_RES
    x = np.asarray(x, dtype=np.float32)
    w_qkv = np.asarray(w_qkv, dtype=np.float32)
    w_fc = np.asarray(w_fc, dtype=np.float32)
    b_fc = np.asarray(b_fc, dtype=np.float32)

    if _NC_CACHE is None:
        _NC_CACHE = _build_program()
    nc = _NC_CACHE

    bf = ml_dtypes.bfloat16
    wvT = np.ascontiguousarray(w_qkv[2 * D:].T).astype(bf)
    wkT = np.ascontiguousarray(w_qkv[D:2 * D].T).astype(bf)
    wq = np.ascontiguousarray(SCALE * w_qkv[:D]).astype(bf)
    wfcT = np.ascontiguousarray(w_fc.T).astype(bf)
    ident = np.eye(128, dtype=bf)

    in_maps = []
    for b in range(B):
        in_maps.append({
            "xN": x[b].astype(bf),
            "xT": np.ascontiguousarray(x[b].T).astype(bf),
            "wvT": wvT, "wkT": wkT, "wq": wq, "wfcT": wfcT,
            "bfc": b_fc, "ident": ident,
        })

    res = bass_utils.run_bass_kernel_spmd(
        nc, in_maps, core_ids=list(range(B)), trace=_trace
    )
    LAST_EXEC_NS = res.exec_time_ns
    LAST_RES = res
    out = np.stack([res.results[b]["outT"].astype(np.float32).T
                    for b in range(B)])
    return np.ascontiguousarray(out)


# revision 8
# speedup vs baseline: 1.5312x; 1.1193x over previous
"""Trainium2 Bass kernel for nn_Attention_84585085927925 — bf16 M-folded Gram.

Reference (per batch element b, all fp32):
    qkv = x @ w_qkv.T ; q,k,v heads of 64 ; attn = sqrt(64) * q @ k.T (NO
    softmax) ; out = attn @ v ; out = out @ w_fc.T + b_fc

No softmax => attention is linear; fold k/v AND the q/fc projections into a
single per-batch effective matrix M:
    out = x @ M + b_fc,   M = s * sum_h wq_h.T G_h wfc[:,h].T,
    G_h = wk_h C wv_h.T,  C = x.T x  (symmetric)
Per-core pipeline (one batch element per NeuronCore, 8 cores):
    C    upper-triangle blocks via PSUM, mirrored by PE transpose
    T1   = C @ wv.T                       [768,768]
    G    = wk_pair @ T1[:,pair]  (block-diag per head pair)
    M1T  = G_blkdiag.T @ (s*wq)_pair      [128,768] per pair
    M    = M1T.T @ wfc.T                  [768,768]
    outT = M.T @ xT + b_fc                [768,1024]
All matmuls bf16 (1 cyc/row, FWL weight loads); fp32 PSUM accumulate.
"""

import numpy as np
import ml_dtypes

import concourse.bass as bass  # noqa: F401  (registers engine namespaces)
import concourse.mybir as mybir
import concourse.tile as tile
from concourse import bacc, bass_utils

F32 = mybir.dt.float32
BF16 = mybir.dt.bfloat16
U32 = mybir.dt.uint32

B, N, D, H = 8, 1024, 768, 12
HD = D // H            # 64
SCALE = float(np.sqrt(HD))
DT = D // 128           # 6 blocks of 128 along feature dims
NT = N // 128           # 8 token tiles


def _build_program(debug_dumps=False, num_devices=B):
    nc = bacc.Bacc(
        trn_type="TRN2", target_bir_lowering=False, debug=False,
        num_devices=num_devices
    )
    xN_d = nc.dram_tensor("xN", [N, D], BF16, kind="ExternalInput").ap()
    xT_d = nc.dram_tensor("xT", [D, N], BF16, kind="ExternalInput").ap()
    wvT_d = nc.dram_tensor("wvT", [D, D], BF16, kind="ExternalInput").ap()
    wkT_d = nc.dram_tensor("wkT", [D, D], BF16, kind="ExternalInput").ap()
    wq_d = nc.dram_tensor("wq", [D, D], BF16, kind="ExternalInput").ap()
    wfcT_d = nc.dram_tensor("wfcT", [D, D], BF16, kind="ExternalInput").ap()
    bfc_d = nc.dram_tensor("bfc", [D], F32, kind="ExternalInput").ap()
    id_d = nc.dram_tensor("ident", [128, 128], BF16, kind="ExternalInput").ap()
    outT_d = nc.dram_tensor("outT", [D, N], BF16, kind="ExternalOutput").ap()
    dbg = {}
    if debug_dumps:
        for nm, shape in [("d_c", [128, DT, D]), ("d_t1", [128, DT, D]),
                          ("d_g2", [128, DT, 128]), ("d_m1t", [128, DT, D]),
                          ("d_m", [128, DT, D])]:
            dbg[nm] = nc.dram_tensor(nm, shape, BF16,
                                     kind="ExternalOutput").ap()

    with tile.TileContext(nc) as tc:
        with tc.tile_pool(name="big", bufs=1) as big, \
             tc.tile_pool(name="outsp", bufs=3) as outsp, \
             tc.tile_pool(name="psa", bufs=4, space="PSUM") as psa, \
             tc.tile_pool(name="psw", bufs=2, space="PSUM") as psw, \
             tc.tile_pool(name="psg", bufs=2, space="PSUM") as psg:

        # PSUM static budget: psa 4 banks + psw 2 + psg 2 = 8.

            xn_t = [big.tile([128, D], BF16, name=f"xn{o}") for o in range(NT)]
            xT_sb = big.tile([128, DT, N], BF16, name="xT_sb")
            wvT_sb = big.tile([128, DT, D], BF16, name="wvT_sb")
            wkT_sb = big.tile([128, DT, D], BF16, name="wkT_sb")
            wq_sb = big.tile([128, DT, D], BF16, name="wq_sb")
            wfcT_sb = big.tile([128, DT, D], BF16, name="wfcT_sb")
            c_sb = big.tile([128, DT, D], BF16, name="c_sb")
            t1_sb = big.tile([128, DT, D], BF16, name="t1_sb")
            g2_sb = big.tile([128, DT, 128], BF16, name="g2_sb")
            m1t_sb = big.tile([128, DT, D], BF16, name="m1t_sb")
            m_sb = big.tile([128, DT, D], BF16, name="m_sb")
            bias_sb = big.tile([128, DT], F32, name="bias_sb")
            id_sb = big.tile([128, 128], BF16, name="id_sb")

            xN_r = xN_d.rearrange("(o p) e -> p o e", p=128)
            xT_r = xT_d.rearrange("(o p) n -> p o n", p=128)
            wvT_r = wvT_d.rearrange("(o p) e -> p o e", p=128)
            wkT_r = wkT_d.rearrange("(o p) e -> p o e", p=128)
            wq_r = wq_d.rearrange("(o p) e -> p o e", p=128)
            wfcT_r = wfcT_d.rearrange("(o p) e -> p o e", p=128)
            outT_r = outT_d.rearrange("(o p) n -> p o n", p=128)

            # ---- DMA in on two hwdge queues (sync + scalar) ----
            # sync queue: xN even tiles first (C starts immediately)
            # scalar queue: xN odd tiles, then remaining weights
            nc.sync.dma_start(xn_t[0][:], xN_r[:, 0, :])
            nc.scalar.dma_start(xn_t[1][:], xN_r[:, 1, :])
            nc.sync.dma_start(xn_t[2][:], xN_r[:, 2, :])
            nc.scalar.dma_start(xn_t[3][:], xN_r[:, 3, :])
            nc.sync.dma_start(xn_t[4][:], xN_r[:, 4, :])
            nc.scalar.dma_start(xn_t[5][:], xN_r[:, 5, :])
            nc.sync.dma_start(xn_t[6][:], xN_r[:, 6, :])
            nc.scalar.dma_start(xn_t[7][:], xN_r[:, 7, :])
            nc.sync.dma_start(id_sb[:], id_d)
            nc.sync.dma_start(bias_sb[:], bfc_d.rearrange("(o p) -> p o", p=128))
            # strict first-needed priority, halves split across both queues
            s0, s1 = slice(0, 3), slice(3, 6)
            nc.sync.dma_start(wvT_sb[:, s0, :], wvT_r[:, s0, :])
            nc.scalar.dma_start(wvT_sb[:, s1, :], wvT_r[:, s1, :])
            nc.sync.dma_start(wkT_sb[:, s0, :], wkT_r[:, s0, :])
            nc.scalar.dma_start(wkT_sb[:, s1, :], wkT_r[:, s1, :])
            nc.sync.dma_start(wq_sb[:, s0, :], wq_r[:, s0, :])
            nc.scalar.dma_start(wq_sb[:, s1, :], wq_r[:, s1, :])
            nc.sync.dma_start(wfcT_sb[:, s0, :], wfcT_r[:, s0, :])
            nc.scalar.dma_start(wfcT_sb[:, s1, :], wfcT_r[:, s1, :])
            nc.sync.dma_start(xT_sb[:, s0, :], xT_r[:, s0, :])
            nc.scalar.dma_start(xT_sb[:, s1, :], xT_r[:, s1, :])

            # zero g2 once (gpsimd — off critical path)
            nc.gpsimd.memset(g2_sb[:], 0.0)

            copy_engines = [nc.vector.tensor_copy, nc.scalar.copy]
            ce_idx = [0]

            def copy(dst, src):
                copy_engines[ce_idx[0] % 2](dst, src)
                ce_idx[0] += 1

            # ---- C = x.T x, upper triangle, 3 phases of 2 rows ----
            # row r covers cols r*128:768 (split into <=512 chunks)
            row_chunks = {0: [(0, 512), (512, 256)], 1: [(128, 512), (640, 128)],
                          2: [(256, 512)], 3: [(384, 384)],
                          4: [(512, 256)], 5: [(640, 128)]}

            def c_phase(rows):
                tiles = []
                for r in rows:
                    for c0, w in row_chunks[r]:
                        pt = psa.tile([128, 512], F32, tag="a", name=f"c{r}_{c0}")
                        tiles.append((r, c0, w, pt))
                for nt in range(NT):
                    for r, c0, w, pt in tiles:
                        nc.tensor.matmul(
                            pt[:, :w],
                            xn_t[nt][:, r * 128:(r + 1) * 128],
                            xn_t[nt][:, c0:c0 + w],
                            start=(nt == 0), stop=(nt == NT - 1),
                        )
                for r, c0, w, pt in tiles:
                    copy(c_sb[:, r, c0:c0 + w], pt[:, :w])

            def emit_mirror(i, j):
                # slot (j, i) := transpose of stored upper block (i, j)
                tp = psg.tile([128, 128], BF16, tag="g", name=f"tr{i}{j}")
                nc.tensor.transpose(
                    tp[:], c_sb[:, i, j * 128:(j + 1) * 128], id_sb[:]
                )
                nc.vector.tensor_copy(
                    c_sb[:, j, i * 128:(i + 1) * 128].bitcast(U32),
                    tp[:].bitcast(U32))

            c_phase([0, 1])
            c_phase([2, 3])
            for i, j in [(0, 1), (0, 2), (0, 3), (0, 4), (0, 5),
                         (1, 2), (1, 3), (1, 4), (1, 5)]:
                emit_mirror(i, j)
            c_phase([4, 5])
            for i, j in [(2, 3), (2, 4), (2, 5), (3, 4), (3, 5), (4, 5)]:
                emit_mirror(i, j)

            # ---- T1 = C @ wv.T ----
            for i in range(DT):
                ua = psa.tile([128, 512], F32, tag="a", name=f"t1a{i}")
                ub = psa.tile([128, 512], F32, tag="a", name=f"t1b{i}")
                for j in range(DT):
                    lhsT = c_sb[:, j, i * 128:(i + 1) * 128]
                    nc.tensor.matmul(ua[:, :512], lhsT, wvT_sb[:, j, 0:512],
                                     start=(j == 0), stop=(j == DT - 1))
                    nc.tensor.matmul(ub[:, :256], lhsT, wvT_sb[:, j, 512:768],
                                     start=(j == 0), stop=(j == DT - 1))
                copy(t1_sb[:, i, 0:512], ua[:, :512])
                copy(t1_sb[:, i, 512:768], ub[:, :256])

            # ---- G (block-diag per pair) + M1T + M, software-pipelined ----
            def g_stage(t):
                gp = psg.tile([128, 128], F32, tag="g", name=f"g{t}")
                for j in range(DT):
                    nc.tensor.matmul(
                        gp[:],
                        wkT_sb[:, j, t * 128:(t + 1) * 128],
                        t1_sb[:, j, t * 128:(t + 1) * 128],
                        start=(j == 0), stop=(j == DT - 1),
                    )
                copy(g2_sb[0:64, t, 0:64], gp[0:64, 0:64])
                copy(g2_sb[64:128, t, 64:128], gp[64:128, 64:128])

            def m1t_stage(t):
                pa = psw.tile([128, 512], F32, tag="w", name=f"m1a{t}")
                pb = psw.tile([128, 512], F32, tag="w", name=f"m1b{t}")
                nc.tensor.matmul(pa[:, :512], g2_sb[:, t, :],
                                 wq_sb[:, t, 0:512], start=True, stop=True)
                nc.tensor.matmul(pb[:, :256], g2_sb[:, t, :],
                                 wq_sb[:, t, 512:768], start=True, stop=True)
                copy(m1t_sb[:, t, 0:512], pa[:, :512])
                copy(m1t_sb[:, t, 512:768], pb[:, :256])

            # M accumulators for dblocks 0,1 live across the G/M1T pipeline
            mtiles01 = []
            for db in (0, 1):
                ma = psa.tile([128, 512], F32, tag="a", name=f"ma{db}")
                mb = psa.tile([128, 512], F32, tag="a", name=f"mb{db}")
                mtiles01.append((db, ma, mb))

            def m_step(dbtiles, t):
                for db, ma, mb in dbtiles:
                    lhsT = m1t_sb[:, t, db * 128:(db + 1) * 128]
                    nc.tensor.matmul(ma[:, :512], lhsT, wfcT_sb[:, t, 0:512],
                                     start=(t == 0), stop=(t == DT - 1))
                    nc.tensor.matmul(mb[:, :256], lhsT, wfcT_sb[:, t, 512:768],
                                     start=(t == 0), stop=(t == DT - 1))

            g_stage(0)
            g_stage(1)
            m1t_stage(0)
            g_stage(2)
            m1t_stage(1)
            m_step(mtiles01, 0)
            g_stage(3)
            m1t_stage(2)
            m_step(mtiles01, 1)
            g_stage(4)
            m1t_stage(3)
            m_step(mtiles01, 2)
            g_stage(5)
            m1t_stage(4)
            m_step(mtiles01, 3)
            m1t_stage(5)
            m_step(mtiles01, 4)
            m_step(mtiles01, 5)
            for db, ma, mb in mtiles01:
                copy(m_sb[:, db, 0:512], ma[:, :512])
                copy(m_sb[:, db, 512:768], mb[:, :256])
            for phase in (1, 2):
                mts = []
                for db in (phase * 2, phase * 2 + 1):
                    ma = psa.tile([128, 512], F32, tag="a", name=f"ma{db}")
                    mb = psa.tile([128, 512], F32, tag="a", name=f"mb{db}")
                    mts.append((db, ma, mb))
                for t in range(DT):
                    m_step(mts, t)
                for db, ma, mb in mts:
                    copy(m_sb[:, db, 0:512], ma[:, :512])
                    copy(m_sb[:, db, 512:768], mb[:, :256])

            # ---- outT = M.T @ xT + bias ----
            for et in range(DT):
                oa = psa.tile([128, 512], F32, tag="a", name=f"oa{et}")
                ob = psa.tile([128, 512], F32, tag="a", name=f"ob{et}")
                for dt in range(DT):
                    lhsT = m_sb[:, dt, et * 128:(et + 1) * 128]
                    nc.tensor.matmul(oa[:, :512], lhsT, xT_sb[:, dt, 0:512],
                                     start=(dt == 0), stop=(dt == DT - 1))
                    nc.tensor.matmul(ob[:, :512], lhsT, xT_sb[:, dt, 512:1024],
                                     start=(dt == 0), stop=(dt == DT - 1))
                ot = outsp.tile([128, N], BF16, tag="ot", name=f"ot{et}")
                if et < DT - 1:
                    nc.scalar.add(ot[:, 0:512], oa[:, :512],
                                  bias_sb[:, et:et + 1])
                    nc.vector.tensor_scalar_add(ot[:, 512:1024], ob[:, :512],
                                                bias_sb[:, et:et + 1])
                    nc.sync.dma_start(outT_r[:, et, :], ot[:])
                else:
                    # last tile: stream quarter-chunks to shorten the tail
                    badd = [nc.scalar.add, nc.vector.tensor_scalar_add]
                    dmae = [nc.sync, nc.scalar]
                    for q in range(4):
                        src = (oa, ob)[q // 2]
                        c0 = (q % 2) * 256
                        lo = q * 256
                        badd[q % 2](ot[:, lo:lo + 256], src[:, c0:c0 + 256],
                                    bias_sb[:, et:et + 1])
                        dmae[q % 2].dma_start(outT_r[:, et, lo:lo + 256],
                                              ot[:, lo:lo + 256])

            if debug_dumps:
                for nm, sb in [("d_c", c_sb), ("d_t1", t1_sb),
                               ("d_g2", g2_sb), ("d_m1t", m1t_sb),
                               ("d_m", m_sb)]:
                    nc.sync.dma_start(dbg[nm], sb)

    nc.compile()
    return nc


_NC_CACHE = None
LAST_EXEC_NS = None
LAST_RES = None


def kernel(x, w_qkv, w_fc, b_fc, _trace=False):
    global _NC_CACHE, LAST_EXEC_NS, LAST_RES
    x = np.asarray(x, dtype=np.float32)
    w_qkv = np.asarray(w_qkv, dtype=np.float32)
    w_fc = np.asarray(w_fc, dtype=np.float32)
    b_fc = np.asarray(b_fc, dtype=np.float32)

    if _NC_CACHE is None:
        _NC_CACHE = _build_program()
    nc = _NC_CACHE

    bf = ml_dtypes.bfloat16
    wvT = np.ascontiguousarray(w_qkv[2 * D:].T).astype(bf)
    wkT = np.ascontiguousarray(w_qkv[D:2 * D].T).astype(bf)
    wq = np.ascontiguousarray(SCALE * w_qkv[:D]).astype(bf)
    wfcT = np.ascontiguousarray(w_fc.T).astype(bf)
    ident = np.eye(128, dtype=bf)

    in_maps = []
    for b in range(B):
        in_maps.append({
            "xN": x[b].astype(bf),
            "xT": np.ascontiguousarray(x[b].T).astype(bf),
            "wvT": wvT, "wkT": wkT, "wq": wq, "wfcT": wfcT,
            "bfc": b_fc, "ident": ident,
        })

    res = bass_utils.run_bass_kernel_spmd(
        nc, in_maps, core_ids=list(range(B)), trace=_trace
    )
    LAST_EXEC_NS = res.exec_time_ns
    LAST_RES = res
    out = np.stack([res.results[b]["outT"].astype(np.float32).T
                    for b in range(B)])
    return np.ascontiguousarray(out)


# revision 20
# speedup vs baseline: 1.5647x; 1.0219x over previous
"""Trainium2 Bass kernel for nn_Attention_84585085927925 — bf16 M-folded Gram.

Reference (per batch element b, all fp32):
    qkv = x @ w_qkv.T ; q,k,v heads of 64 ; attn = sqrt(64) * q @ k.T (NO
    softmax) ; out = attn @ v ; out = out @ w_fc.T + b_fc

No softmax => attention is linear; fold k/v AND the q/fc projections into a
single per-batch effective matrix M:
    out = x @ M + b_fc,   M = s * sum_h wq_h.T G_h wfc[:,h].T,
    G_h = wk_h C wv_h.T,  C = x.T x  (symmetric)
Per-core pipeline (one batch element per NeuronCore, 8 cores):
    C    upper-triangle blocks via PSUM, mirrored by PE transpose
    T1   = C @ wv.T                       [768,768]
    G    = wk_pair @ T1[:,pair]  (block-diag per head pair)
    M1T  = G_blkdiag.T @ (s*wq)_pair      [128,768] per pair
    M    = M1T.T @ wfc.T                  [768,768]
    outT = M.T @ xT + b_fc                [768,1024]
All matmuls bf16 (1 cyc/row, FWL weight loads); fp32 PSUM accumulate.
"""

import numpy as np
import ml_dtypes

import concourse.bass as bass  # noqa: F401  (registers engine namespaces)
import concourse.mybir as mybir
import concourse.tile as tile
from concourse import bacc, bass_utils

F32 = mybir.dt.float32
BF16 = mybir.dt.bfloat16
U32 = mybir.dt.uint32

B, N, D, H = 8, 1024, 768, 12
HD = D // H            # 64
SCALE = float(np.sqrt(HD))
DT = D // 128           # 6 blocks of 128 along feature dims
NT = N // 128           # 8 token tiles


def _build_program(debug_dumps=False, num_devices=B):
    nc = bacc.Bacc(
        trn_type="TRN2", target_bir_lowering=False, debug=False,
        num_devices=num_devices
    )
    xN_d = nc.dram_tensor("xN", [N, D], BF16, kind="ExternalInput").ap()
    xT_d = nc.dram_tensor("xT", [D, N], BF16, kind="ExternalInput").ap()
    wvT_d = nc.dram_tensor("wvT", [D, D], BF16, kind="ExternalInput").ap()
    wkT_d = nc.dram_tensor("wkT", [D, D], BF16, kind="ExternalInput").ap()
    wq_d = nc.dram_tensor("wq", [D, D], BF16, kind="ExternalInput").ap()
    wfcT_d = nc.dram_tensor("wfcT", [D, D], BF16, kind="ExternalInput").ap()
    bfc_d = nc.dram_tensor("bfc", [D], F32, kind="ExternalInput").ap()
    id_d = nc.dram_tensor("ident", [128, 128], BF16, kind="ExternalInput").ap()
    outT_d = nc.dram_tensor("outT", [D, N], BF16, kind="ExternalOutput").ap()
    dbg = {}
    if debug_dumps:
        for nm, shape in [("d_c", [128, DT, D]), ("d_t1", [128, DT, D]),
                          ("d_g2", [128, DT, 128]), ("d_m1t", [128, DT, D]),
                          ("d_m", [128, DT, D])]:
            dbg[nm] = nc.dram_tensor(nm, shape, BF16,
                                     kind="ExternalOutput").ap()

    with tile.TileContext(nc) as tc:
        with tc.tile_pool(name="big", bufs=1) as big, \
             tc.tile_pool(name="outsp", bufs=3) as outsp, \
             tc.tile_pool(name="psa", bufs=4, space="PSUM") as psa, \
             tc.tile_pool(name="psw", bufs=2, space="PSUM") as psw, \
             tc.tile_pool(name="psg", bufs=2, space="PSUM") as psg:

        # PSUM static budget: psa 4 banks + psw 2 + psg 2 = 8.

            xn_t = [big.tile([128, D], BF16, name=f"xn{o}") for o in range(NT)]
            xT_sb = big.tile([128, DT, N], BF16, name="xT_sb")
            wvT_sb = big.tile([128, DT, D], BF16, name="wvT_sb")
            wkT_sb = big.tile([128, DT, D], BF16, name="wkT_sb")
            wq_sb = big.tile([128, DT, D], BF16, name="wq_sb")
            wfcT_sb = big.tile([128, DT, D], BF16, name="wfcT_sb")
            c_sb = big.tile([128, DT, D], BF16, name="c_sb")
            t1_sb = big.tile([128, DT, D], BF16, name="t1_sb")
            g2_sb = big.tile([128, DT, 128], BF16, name="g2_sb")
            m1t_sb = big.tile([128, DT, D], BF16, name="m1t_sb")
            m_sb = big.tile([128, DT, D], BF16, name="m_sb")
            bias_sb = big.tile([128, DT], F32, name="bias_sb")
            id_sb = big.tile([128, 128], BF16, name="id_sb")
            scr_sb = big.tile([128, 512], BF16, name="scr_sb")

            xN_r = xN_d.rearrange("(o p) e -> p o e", p=128)
            xT_r = xT_d.rearrange("(o p) n -> p o n", p=128)
            wvT_r = wvT_d.rearrange("(o p) e -> p o e", p=128)
            wkT_r = wkT_d.rearrange("(o p) e -> p o e", p=128)
            wq_r = wq_d.rearrange("(o p) e -> p o e", p=128)
            wfcT_r = wfcT_d.rearrange("(o p) e -> p o e", p=128)
            outT_r = outT_d.rearrange("(o p) n -> p o n", p=128)

            # ---- DMA in on two hwdge queues (sync + scalar) ----
            # sync queue: xN even tiles first (C starts immediately)
            # scalar queue: xN odd tiles, then remaining weights
            nc.sync.dma_start(xn_t[0][:], xN_r[:, 0, :])
            nc.scalar.dma_start(xn_t[1][:], xN_r[:, 1, :])
            nc.sync.dma_start(xn_t[2][:], xN_r[:, 2, :])
            nc.scalar.dma_start(xn_t[3][:], xN_r[:, 3, :])
            nc.sync.dma_start(xn_t[4][:], xN_r[:, 4, :])
            nc.scalar.dma_start(xn_t[5][:], xN_r[:, 5, :])
            nc.sync.dma_start(xn_t[6][:], xN_r[:, 6, :])
            nc.scalar.dma_start(xn_t[7][:], xN_r[:, 7, :])
            nc.sync.dma_start(id_sb[:], id_d)
            nc.sync.dma_start(bias_sb[:], bfc_d.rearrange("(o p) -> p o", p=128))
            # strict first-needed priority, halves split across both queues
            s0, s1 = slice(0, 3), slice(3, 6)
            nc.sync.dma_start(wvT_sb[:, s0, :], wvT_r[:, s0, :])
            nc.scalar.dma_start(wvT_sb[:, s1, :], wvT_r[:, s1, :])
            nc.sync.dma_start(wkT_sb[:, s0, :], wkT_r[:, s0, :])
            nc.scalar.dma_start(wkT_sb[:, s1, :], wkT_r[:, s1, :])
            nc.sync.dma_start(wq_sb[:, s0, :], wq_r[:, s0, :])
            nc.scalar.dma_start(wq_sb[:, s1, :], wq_r[:, s1, :])
            nc.sync.dma_start(wfcT_sb[:, s0, :], wfcT_r[:, s0, :])
            nc.scalar.dma_start(wfcT_sb[:, s1, :], wfcT_r[:, s1, :])
            nc.sync.dma_start(xT_sb[:, s0, :], xT_r[:, s0, :])
            nc.scalar.dma_start(xT_sb[:, s1, :], xT_r[:, s1, :])

            # zero g2 once (gpsimd — off critical path)
            nc.gpsimd.memset(g2_sb[:], 0.0)

            # warm up the PE p-state while the first xN DMA is in flight
            nc.vector.memset(scr_sb[:], 0.0)
            for k in range(8):
                wu = psw.tile([128, 512], F32, tag="w", name=f"wu{k}")
                nc.tensor.matmul(wu[:, :512], scr_sb[:, 0:128],
                                 scr_sb[:, :512], start=True, stop=True)

            copy_engines = [nc.vector.tensor_copy, nc.scalar.copy]
            ce_idx = [0]

            def copy(dst, src):
                copy_engines[ce_idx[0] % 2](dst, src)
                ce_idx[0] += 1

            # ---- C = x.T x, upper triangle, 3 phases of 2 rows ----
            # row r covers cols r*128:768 (split into <=512 chunks)
            row_chunks = {0: [(0, 512), (512, 256)], 1: [(128, 512), (640, 128)],
                          2: [(256, 512)], 3: [(384, 384)],
                          4: [(512, 256)], 5: [(640, 128)]}

            def c_phase(rows):
                tiles = []
                for r in rows:
                    for c0, w in row_chunks[r]:
                        pt = psa.tile([128, 512], F32, tag="a", name=f"c{r}_{c0}")
                        tiles.append((r, c0, w, pt))
                for nt in range(NT):
                    for r, c0, w, pt in tiles:
                        nc.tensor.matmul(
                            pt[:, :w],
                            xn_t[nt][:, r * 128:(r + 1) * 128],
                            xn_t[nt][:, c0:c0 + w],
                            start=(nt == 0), stop=(nt == NT - 1),
                        )
                for r, c0, w, pt in tiles:
                    copy(c_sb[:, r, c0:c0 + w], pt[:, :w])

            def emit_mirror(i, j):
                # slot (j, i) := transpose of stored upper block (i, j)
                tp = psg.tile([128, 128], BF16, tag="g", name=f"tr{i}{j}")
                nc.tensor.transpose(
                    tp[:], c_sb[:, i, j * 128:(j + 1) * 128], id_sb[:]
                )
                nc.vector.tensor_copy(
                    c_sb[:, j, i * 128:(i + 1) * 128].bitcast(U32),
                    tp[:].bitcast(U32))

            c_phase([0, 1])
            c_phase([2, 3])
            for i, j in [(0, 1), (0, 2), (0, 3), (0, 4), (0, 5),
                         (1, 2), (1, 3), (1, 4), (1, 5)]:
                emit_mirror(i, j)
            c_phase([4, 5])
            for i, j in [(2, 3), (2, 4), (2, 5), (3, 4), (3, 5), (4, 5)]:
                emit_mirror(i, j)

            # ---- T1 = C @ wv.T ----
            for i in range(DT):
                ua = psa.tile([128, 512], F32, tag="a", name=f"t1a{i}")
                ub = psa.tile([128, 512], F32, tag="a", name=f"t1b{i}")
                for j in range(DT):
                    lhsT = c_sb[:, j, i * 128:(i + 1) * 128]
                    nc.tensor.matmul(ua[:, :512], lhsT, wvT_sb[:, j, 0:512],
                                     start=(j == 0), stop=(j == DT - 1))
                    nc.tensor.matmul(ub[:, :256], lhsT, wvT_sb[:, j, 512:768],
                                     start=(j == 0), stop=(j == DT - 1))
                copy(t1_sb[:, i, 0:512], ua[:, :512])
                copy(t1_sb[:, i, 512:768], ub[:, :256])

            # ---- G (block-diag per pair) + M1T + M, software-pipelined ----
            def g_stage(t):
                gp = psg.tile([128, 128], F32, tag="g", name=f"g{t}")
                for j in range(DT):
                    nc.tensor.matmul(
                        gp[:],
                        wkT_sb[:, j, t * 128:(t + 1) * 128],
                        t1_sb[:, j, t * 128:(t + 1) * 128],
                        start=(j == 0), stop=(j == DT - 1),
                    )
                copy(g2_sb[0:64, t, 0:64], gp[0:64, 0:64])
                copy(g2_sb[64:128, t, 64:128], gp[64:128, 64:128])

            def m1t_stage(t):
                pa = psw.tile([128, 512], F32, tag="w", name=f"m1a{t}")
                pb = psw.tile([128, 512], F32, tag="w", name=f"m1b{t}")
                nc.tensor.matmul(pa[:, :512], g2_sb[:, t, :],
                                 wq_sb[:, t, 0:512], start=True, stop=True)
                nc.tensor.matmul(pb[:, :256], g2_sb[:, t, :],
                                 wq_sb[:, t, 512:768], start=True, stop=True)
                copy(m1t_sb[:, t, 0:512], pa[:, :512])
                copy(m1t_sb[:, t, 512:768], pb[:, :256])

            # M accumulators for dblocks 0,1 live across the G/M1T pipeline
            mtiles01 = []
            for db in (0, 1):
                ma = psa.tile([128, 512], F32, tag="a", name=f"ma{db}")
                mb = psa.tile([128, 512], F32, tag="a", name=f"mb{db}")
                mtiles01.append((db, ma, mb))

            def m_step(dbtiles, t):
                for db, ma, mb in dbtiles:
                    lhsT = m1t_sb[:, t, db * 128:(db + 1) * 128]
                    nc.tensor.matmul(ma[:, :512], lhsT, wfcT_sb[:, t, 0:512],
                                     start=(t == 0), stop=(t == DT - 1))
                    nc.tensor.matmul(mb[:, :256], lhsT, wfcT_sb[:, t, 512:768],
                                     start=(t == 0), stop=(t == DT - 1))

            g_stage(0)
            g_stage(1)
            m1t_stage(0)
            g_stage(2)
            m1t_stage(1)
            m_step(mtiles01, 0)
            g_stage(3)
            m1t_stage(2)
            m_step(mtiles01, 1)
            g_stage(4)
            m1t_stage(3)
            m_step(mtiles01, 2)
            g_stage(5)
            m1t_stage(4)
            m_step(mtiles01, 3)
            m1t_stage(5)
            m_step(mtiles01, 4)
            m_step(mtiles01, 5)
            for db, ma, mb in mtiles01:
                copy(m_sb[:, db, 0:512], ma[:, :512])
                copy(m_sb[:, db, 512:768], mb[:, :256])
            for phase in (1, 2):
                mts = []
                for db in (phase * 2, phase * 2 + 1):
                    ma = psa.tile([128, 512], F32, tag="a", name=f"ma{db}")
                    mb = psa.tile([128, 512], F32, tag="a", name=f"mb{db}")
                    mts.append((db, ma, mb))
                for t in range(DT):
                    m_step(mts, t)
                for db, ma, mb in mts:
                    copy(m_sb[:, db, 0:512], ma[:, :512])
                    copy(m_sb[:, db, 512:768], mb[:, :256])

            # ---- outT = M.T @ xT + bias ----
            for et in range(DT):
                oa = psa.tile([128, 512], F32, tag="a", name=f"oa{et}")
                ob = psa.tile([128, 512], F32, tag="a", name=f"ob{et}")
                ot = outsp.tile([128, N], BF16, tag="ot", name=f"ot{et}")
                if et < DT - 1:
                    for dt in range(DT):
                        lhsT = m_sb[:, dt, et * 128:(et + 1) * 128]
                        nc.tensor.matmul(oa[:, :512], lhsT,
                                         xT_sb[:, dt, 0:512],
                                         start=(dt == 0), stop=(dt == DT - 1))
                        nc.tensor.matmul(ob[:, :512], lhsT,
                                         xT_sb[:, dt, 512:1024],
                                         start=(dt == 0), stop=(dt == DT - 1))
                    nc.scalar.add(ot[:, 0:512], oa[:, :512],
                                  bias_sb[:, et:et + 1])
                    nc.vector.tensor_scalar_add(ot[:, 512:1024], ob[:, :512],
                                                bias_sb[:, et:et + 1])
                    nc.sync.dma_start(outT_r[:, et, :], ot[:])
                else:
                    # last tile: finish the low n-half first so its bias+DMA
                    # overlaps the high half's matmuls, shortening the tail
                    for dt in range(DT):
                        nc.tensor.matmul(oa[:, :512],
                                         m_sb[:, dt, et * 128:(et + 1) * 128],
                                         xT_sb[:, dt, 0:512],
                                         start=(dt == 0), stop=(dt == DT - 1))
                    nc.scalar.add(ot[:, 0:256], oa[:, 0:256],
                                  bias_sb[:, et:et + 1])
                    nc.sync.dma_start(outT_r[:, et, 0:256], ot[:, 0:256])
                    nc.vector.tensor_scalar_add(ot[:, 256:512], oa[:, 256:512],
                                                bias_sb[:, et:et + 1])
                    nc.scalar.dma_start(outT_r[:, et, 256:512], ot[:, 256:512])
                    for dt in range(DT):
                        nc.tensor.matmul(ob[:, :512],
                                         m_sb[:, dt, et * 128:(et + 1) * 128],
                                         xT_sb[:, dt, 512:1024],
                                         start=(dt == 0), stop=(dt == DT - 1))
                    nc.scalar.add(ot[:, 512:768], ob[:, 0:256],
                                  bias_sb[:, et:et + 1])
                    nc.sync.dma_start(outT_r[:, et, 512:768], ot[:, 512:768])
                    nc.vector.tensor_scalar_add(ot[:, 768:1024],
                                                ob[:, 256:512],
                                                bias_sb[:, et:et + 1])
                    nc.scalar.dma_start(outT_r[:, et, 768:1024],
                                        ot[:, 768:1024])

            if debug_dumps:
                for nm, sb in [("d_c", c_sb), ("d_t1", t1_sb),
                               ("d_g2", g2_sb), ("d_m1t", m1t_sb),
                               ("d_m", m_sb)]:
                    nc.sync.dma_start(dbg[nm], sb)

    nc.compile()
    return nc


_NC_CACHE = None
LAST_EXEC_NS = None
LAST_RES = None


def kernel(x, w_qkv, w_fc, b_fc, _trace=False):
    global _NC_CACHE, LAST_EXEC_NS, LAST_RES
    x = np.asarray(x, dtype=np.float32)
    w_qkv = np.asarray(w_qkv, dtype=np.float32)
    w_fc = np.asarray(w_fc, dtype=np.float32)
    b_fc = np.asarray(b_fc, dtype=np.float32)

    if _NC_CACHE is None:
        _NC_CACHE = _build_program()
    nc = _NC_CACHE

    bf = ml_dtypes.bfloat16
    wvT = np.ascontiguousarray(w_qkv[2 * D:].T).astype(bf)
    wkT = np.ascontiguousarray(w_qkv[D:2 * D].T).astype(bf)
    wq = np.ascontiguousarray(SCALE * w_qkv[:D]).astype(bf)
    wfcT = np.ascontiguousarray(w_fc.T).astype(bf)
    ident = np.eye(128, dtype=bf)

    in_maps = []
    for b in range(B):
        in_maps.append({
            "xN": x[b].astype(bf),
            "xT": np.ascontiguousarray(x[b].T).astype(bf),
            "wvT": wvT, "wkT": wkT, "wq": wq, "wfcT": wfcT,
            "bfc": b_fc, "ident": ident,
        })

    res = bass_utils.run_bass_kernel_spmd(
        nc, in_maps, core_ids=list(range(B)), trace=_trace
    )
    LAST_EXEC_NS = res.exec_time_ns
    LAST_RES = res
    out = np.stack([res.results[b]["outT"].astype(np.float32).T
                    for b in range(B)])
    return np.ascontiguousarray(out)
